# revision 20
# baseline (speedup 1.0000x reference)
"""Trainium2 Bass kernel for nn_MultiHeadAttention_41944650612760.

Wasserstein-distance multi-head attention with cumulative position decay.
Sharding: data-parallel over batch B=8 across 8 NeuronCores (one batch/core).

Per-core pipeline (T=1024, D=512, H=8, dk=64), in [t-part, s-free] layout:
  P1  six linear projections on PE in fp16 (1 cyc/col); q/k score operands
      kept transposed ([dout, t]) and head-stacked (parity split) so the
      score matmul contracts K=128; v projections normal ([s, dout]) fp16.
  P2  row/col norm terms: a1 -> per-(tb,h) bias scalars; b1 -> a [1, 8, T]
      SBUF row added into score PSUM via K=1 augment matmuls (no broadcast,
      no DRAM bounce).  Causal -inf diag block added via fp16 identity
      matmul augment.
  P3  per head: stage1 streams row-blocks tb=0..7 through
      scores matmul fp16 -> PSUM; {e = Exp(0.125*ps + a1bias) (ACT, fused
      evict), sc16 = (ps - a1)*0.125 fp16 (DVE, frees PSUM)}; C = cumsum
      (DVE scan fp32); t1 = C - sm1 (DVE 2x); dg = t1*|t-s| fp16 (Pool).
      stage2 batches ACT per head so the activation table loads only twice
      per head: [sqrt(rg*dg) x8][te = Exp(-w); p2 = Exp(sc*te)+accum x8],
      with arg = sc16*te16 as a 2x fp16 DVE TT in between; p2 normalized by
      1/sm2 (DVE fp16), DMA-xbar transpose; PV fp16 with p2^2 strips on DVE.
  P4  fp16 output projections with bias via K=1 ones-augment.

zero_pad is applied on the host (row 0 of each output = bias).
"""

import os
import math
import numpy as np
from contextlib import ExitStack

B, T, D, H = 8, 1024, 512, 8
DK = D // H          # 64
NT = T // 128        # 8 row/col blocks
NEG16 = -60000.0     # fp16-safe causal mask add (see analysis: te=1 on
                     # masked lanes since dg=0 there, so exp(-60000*0.125*te')=0)
LN8INV = math.log(0.125)
F16 = np.float16

# packed causal layout for transposed attention weights: block j holds
# t in [j*128, 1024) -> width (8-j)*128, at column offset OFF[j]
OFF = [0] * NT
for _j in range(1, NT):
    OFF[_j] = OFF[_j - 1] + (NT - (_j - 1)) * 128
P2T_COLS = OFF[-1] + 128  # 4608
# packed row-block layout: block tb holds s in [0, (tb+1)*128) at NOFF[tb]
NOFF = [0] * NT
for _t in range(1, NT):
    NOFF[_t] = NOFF[_t - 1] + _t * 128
NN_COLS = NOFF[-1] + NT * 128  # 4608


def _build(gamma2):
    """Trace the Bass program. gamma2[h] = gamma_h**2 (trace-time floats)."""
    import concourse.bass as bass
    import concourse.bacc as bacc
    import concourse.mybir as mybir
    import concourse.tile as tile

    dt = mybir.dt
    AF = mybir.ActivationFunctionType
    OP = mybir.AluOpType
    ts = bass.ts

    nc = bacc.Bacc()

    # ---- per-core DRAM I/O ----
    xT = nc.declare_dram_parameter("xT", [6, D, T], dt.float32, isOutput=False)
    wT = nc.declare_dram_parameter("wT", [4, D, D], dt.float16, isOutput=False)
    woT = nc.declare_dram_parameter("woT", [2, D, D], dt.float16, isOutput=False)
    wc = nc.declare_dram_parameter("wc", [D, H], dt.float16, isOutput=False)
    bqk = nc.declare_dram_parameter("bqk", [128, 12], dt.float32, isOutput=False)
    bvp = nc.declare_dram_parameter("bvp", [128, 8], dt.float32, isOutput=False)
    bvo16 = nc.declare_dram_parameter("bvo16", [2, D], dt.float16, isOutput=False)
    cbr = nc.declare_dram_parameter("cbr", [1, 8], dt.float32, isOutput=False)
    btri = nc.declare_dram_parameter("btri", [128, 128], dt.float16, isOutput=False)
    eye = nc.declare_dram_parameter("eye", [128, 128], dt.float16, isOutput=False)
    selp = nc.declare_dram_parameter("selp", [8, 8, 128], dt.float16, isOutput=False)
    nneg = nc.declare_dram_parameter("nneg", [128, NN_COLS], dt.float16, isOutput=False)
    out_m = nc.declare_dram_parameter("out_m", [T, D], dt.float32, isOutput=True)
    out_c = nc.declare_dram_parameter("out_c", [T, D], dt.float32, isOutput=True)

    with tile.TileContext(nc) as tc, ExitStack() as ctx:
        pc = ctx.enter_context(tc.tile_pool(name="pc", bufs=1))
        pdr = ctx.enter_context(tc.tile_pool(name="pdr", bufs=1, space="DRAM"))

        # ---- persistent SBUF tensors ----
        U2 = pc.tile([128, H, T], dt.float16)       # [qm_h ; sqq_h] per head (parity split)
        W2 = pc.tile([128, H, T], dt.float16)       # [2km_h ; 2sqk_h]
        vm16 = pc.tile([128, NT, D], dt.float16)    # vm normal layout fp16
        vc16 = pc.tile([128, NT, D], dt.float16)
        nneg_sb = pc.tile([128, NN_COLS], dt.float16)
        btri_sb = pc.tile([128, 128], dt.float16)
        eye_sb = pc.tile([128, 128], dt.float16)
        ones16 = pc.tile([1, T], dt.float16)
        a1n8 = pc.tile([128, NT, 8], dt.float32)    # -0.125 * a1tot[t] per (tb, h)
        a1m8 = pc.tile([128, NT, 8], dt.float32)    # -a1tot[t] per (tb, h)
        cbn8 = pc.tile([128, 8], dt.float32)        # -0.125 * cbias, bcast to 128 parts
        bqk_sb = pc.tile([128, 12], dt.float32)     # [bk|bkc_sw|2bk] x 4 chunks
        bvp_sb = pc.tile([128, 8], dt.float32)      # bv, bvc pair-sliced
        wc_sb = pc.tile([128, 4, H], dt.float16)
        E2q = pc.tile([128, 4, 8], dt.float16)
        E2k = pc.tile([128, 4, 8], dt.float16)
        sb8 = pc.tile([8, T], dt.float16)           # -b1 rows (centered)
        sel = pc.tile([8, 8, 128], dt.float16)      # sel[:, h, :]: row h = ones

        nc.sync.dma_start(out=nneg_sb, in_=nneg[:, :])
        nc.sync.dma_start(out=btri_sb, in_=btri[:, :])
        nc.sync.dma_start(out=eye_sb, in_=eye[:, :])
        nc.sync.dma_start(out=bqk_sb, in_=bqk[:, :])
        nc.sync.dma_start(out=bvp_sb, in_=bvp[:, :])
        nc.sync.dma_start(out=wc_sb, in_=wc.rearrange("(k p) h -> p k h", p=128))
        nc.sync.dma_start(out=sel, in_=selp[:, :, :])
        nc.vector.memset(ones16, 1.0)
        _cbr = cbr[0:1, :]
        nc.sync.dma_start(out=cbn8, in_=bass.AP(tensor=_cbr.tensor, offset=_cbr.offset,
                                                ap=[[0, 128]] + list(_cbr.ap[1:])))
        nc.vector.memset(E2q, 0.0)
        nc.vector.memset(E2k, 0.0)
        for c in range(4):
            nc.vector.memset(E2q[0:64, c, 2 * c:2 * c + 1], 1.0)
            nc.vector.memset(E2q[64:128, c, 2 * c + 1:2 * c + 2], 1.0)
            nc.vector.memset(E2k[0:64, c, 2 * c:2 * c + 1], 0.25)
            nc.vector.memset(E2k[64:128, c, 2 * c + 1:2 * c + 2], 0.25)

        # =================== P1: projections + P2: a1/b1 ===================
        with tc.tile_pool(name="p1x", bufs=6) as px, \
             tc.tile_pool(name="p1w", bufs=1) as pw, \
             tc.tile_pool(name="p1z", bufs=3) as pz, \
             tc.tile_pool(name="p1ps", bufs=2, space="PSUM") as pps, \
             tc.tile_pool(name="p1pa", bufs=1, space="PSUM") as ppa:

            b_ps = ppa.tile([8, T], dt.float32, tag="b_ps")   # b1 = m2sq + kcs
            a1t_ps = ppa.tile([128, NT, 8], dt.float32, tag="a1t")  # a1 in [t, (tb,h)]

            def load_x(i):
                xs = []
                for k in range(4):
                    xt = px.tile([128, T], dt.float32, tag="xt")
                    nc.sync.dma_start(out=xt, in_=xT[i, ts(k, 128), :])
                    x16 = px.tile([128, T], dt.float16, tag="x16")
                    nc.vector.tensor_copy(x16, xt)
                    xs.append(x16)
                return xs

            def load_w(i, tagslot):
                wt = pw.tile([128, 4, D], dt.float16, tag=f"wt{tagslot}")
                nc.sync.dma_start(out=wt, in_=wT[i].rearrange("(k p) d -> p k d", p=128))
                return wt

            # ---- run 1: qm -> U2 (bias bk) ----
            xq = load_x(0)
            wk = load_w(0, 0)
            wkc = load_w(1, 1)
            for c in range(4):
                for n in range(2):
                    ps = pps.tile([128, 512], dt.float32, tag="ps")
                    for k in range(4):
                        nc.tensor.matmul(ps, wk[:, k, ts(c, 128)],
                                         xq[k][:, ts(n, 512)],
                                         start=(k == 0), stop=(k == 3))
                    nc.scalar.activation(out=U2[0:64, 2 * c, ts(n, 512)], in_=ps[0:64],
                                         func=AF.Identity, bias=bqk_sb[0:64, c:c + 1])
                    nc.scalar.activation(out=U2[64:128, 2 * c + 1, ts(n, 512)], in_=ps[64:128],
                                         func=AF.Identity, bias=bqk_sb[64:128, c:c + 1])

            # ---- run 2: sqq -> U2 (clip+sqrt) + qcs into a_ps ----
            xqc = load_x(1)
            for c in range(4):
                for n in range(2):
                    ps = pps.tile([128, 512], dt.float32, tag="ps")
                    for k in range(4):
                        nc.tensor.matmul(ps, wkc[:, k, ts(c, 128)],
                                         xqc[k][:, ts(n, 512)],
                                         start=(k == 0), stop=(k == 3))
                    nc.vector.tensor_scalar(out=ps, in0=ps, scalar1=bqk_sb[:, 4 + c:5 + c],
                                            scalar2=1e-24, op0=OP.add, op1=OP.max)
                    # swapped col order: psum[0:64] = head 2c+1 (odd -> low parts)
                    nc.scalar.activation(out=U2[0:64, 2 * c + 1, ts(n, 512)], in_=ps[0:64],
                                         func=AF.Sqrt)
                    nc.scalar.activation(out=U2[64:128, 2 * c, ts(n, 512)], in_=ps[64:128],
                                         func=AF.Sqrt)

            # ---- q-side squares + m1sq + qcs matmuls into a1T form ----
            # one full accumulation group per tb (bank-level group tracking)
            zqs = []
            for c in range(4):
                zq = pz.tile([128, T], dt.float16, tag=f"zq{c}", name="zq", bufs=1)
                nc.scalar.activation(out=zq[0:64, :], in_=U2[0:64, 2 * c, :], func=AF.Square)
                nc.scalar.activation(out=zq[64:128, :], in_=U2[64:128, 2 * c + 1, :], func=AF.Square)
                zqs.append(zq)
            for tb in range(NT):
                for k in range(4):
                    nc.tensor.matmul(a1t_ps[:, tb, :], xqc[k][:, ts(tb, 128)],
                                     wc_sb[:, k, :],
                                     start=(k == 0), stop=False)
                for c in range(4):
                    nc.tensor.matmul(a1t_ps[:, tb, :], zqs[c][:, ts(tb, 128)],
                                     E2q[:, c, :],
                                     start=False, stop=(c == 3))
            nc.scalar.activation(out=a1n8, in_=a1t_ps, func=AF.Copy, scale=-0.125)
            nc.vector.tensor_tensor(out=a1n8, in0=a1n8,
                                    in1=bass.AP(tensor=cbn8.tensor, offset=cbn8.offset,
                                                ap=[cbn8.ap[0], [0, NT]] + list(cbn8.ap[1:])),
                                    op=OP.add)

            # ---- run 3: 2km -> W2 (bias 2bk, scale 2) ----
            xk = load_x(2)
            for c in range(4):
                for n in range(2):
                    ps = pps.tile([128, 512], dt.float32, tag="ps")
                    for k in range(4):
                        nc.tensor.matmul(ps, wk[:, k, ts(c, 128)],
                                         xk[k][:, ts(n, 512)],
                                         start=(k == 0), stop=(k == 3))
                    nc.scalar.activation(out=W2[0:64, 2 * c, ts(n, 512)], in_=ps[0:64],
                                         func=AF.Identity, scale=2.0, bias=bqk_sb[0:64, 8 + c:9 + c])
                    nc.scalar.activation(out=W2[64:128, 2 * c + 1, ts(n, 512)], in_=ps[64:128],
                                         func=AF.Identity, scale=2.0, bias=bqk_sb[64:128, 8 + c:9 + c])

            # ---- run 4: 2sqk -> W2 + kcs into b_ps ----
            xkc = load_x(3)
            for n in range(2):
                for k in range(4):
                    nc.tensor.matmul(b_ps[:, ts(n, 512)], wc_sb[:, k, :],
                                     xkc[k][:, ts(n, 512)],
                                     start=(k == 0), stop=False)
            for c in range(4):
                for n in range(2):
                    ps = pps.tile([128, 512], dt.float32, tag="ps")
                    for k in range(4):
                        nc.tensor.matmul(ps, wkc[:, k, ts(c, 128)],
                                         xkc[k][:, ts(n, 512)],
                                         start=(k == 0), stop=(k == 3))
                    nc.vector.tensor_scalar(out=ps, in0=ps, scalar1=bqk_sb[:, 4 + c:5 + c],
                                            scalar2=1e-24, op0=OP.add, op1=OP.max)
                    nc.scalar.activation(out=W2[0:64, 2 * c + 1, ts(n, 512)], in_=ps[0:64],
                                         func=AF.Sqrt, scale=4.0)
                    nc.scalar.activation(out=W2[64:128, 2 * c, ts(n, 512)], in_=ps[64:128],
                                         func=AF.Sqrt, scale=4.0)

            # ---- k-side squares (of 2km; E2k carries the 1/4) ----
            for c in range(4):
                zk = pz.tile([128, T], dt.float16, tag="z", bufs=2)
                nc.vector.tensor_mul(zk[0:64, :], W2[0:64, 2 * c, :], W2[0:64, 2 * c, :])
                nc.vector.tensor_mul(zk[64:128, :], W2[64:128, 2 * c + 1, :], W2[64:128, 2 * c + 1, :])
                for n in range(2):
                    nc.tensor.matmul(b_ps[:, ts(n, 512)], E2k[:, c, :],
                                     zk[:, ts(n, 512)],
                                     start=False, stop=(c == 3))
            # center b1 rows to keep the fp16 augment rows small:
            # stg_b = -(b_ps - mean_s(b_ps)); the mean goes into a1 via crow8
            mred = pz.tile([8, 1], dt.float32, tag="mred", bufs=1)
            nc.vector.tensor_reduce(out=mred, in_=b_ps, axis=mybir.AxisListType.X,
                                    op=OP.add)
            nc.vector.tensor_scalar(out=mred, in0=mred, scalar1=1.0 / T, scalar2=None,
                                    op0=OP.mult)
            nc.scalar.activation(out=sb8, in_=b_ps, func=AF.Identity,
                                 scale=-1.0, bias=mred)
            crow = pz.tile([1, 8], dt.float32, tag="crow", bufs=1)
            nc.sync.dma_start(out=crow[0:1, :], in_=mred)
            crow8 = pz.tile([1, 8], dt.float32, tag="crow8", bufs=1)
            nc.vector.tensor_scalar(out=crow8, in0=crow, scalar1=-0.125, scalar2=None,
                                    op0=OP.mult)
            crow8b = pz.tile([128, 8], dt.float32, tag="crow8b", bufs=1)
            nc.gpsimd.partition_broadcast(crow8b, crow8)
            nc.vector.tensor_tensor(out=a1n8, in0=a1n8,
                                    in1=bass.AP(tensor=crow8b.tensor, offset=crow8b.offset,
                                                ap=[crow8b.ap[0], [0, NT]] + list(crow8b.ap[1:])),
                                    op=OP.add)
            nc.vector.tensor_scalar(out=a1m8, in0=a1n8, scalar1=8.0, scalar2=None,
                                    op0=OP.mult)

            # ---- runs 5/6: vm, vc (normal layout, fp16) ----
            for i, (xi, wi, dest) in enumerate([(4, 2, vm16), (5, 3, vc16)]):
                xv = load_x(xi)
                wv = load_w(wi, i % 2)
                for m in range(NT):
                    ps = pps.tile([128, 512], dt.float32, tag="ps")
                    for k in range(4):
                        nc.tensor.matmul(ps, xv[k][:, ts(m, 128)],
                                         wv[:, k, :],
                                         start=(k == 0), stop=(k == 3))
                    nc.scalar.activation(out=dest[:, m, :], in_=ps, func=AF.Copy)

        # =================== P3: attention (heads software-pipelined) ======
        # stage1(h): scores->e/sc16->scan->tail->dg   (scan-paced stream)
        # stage2(h-1): sqrt batch, te/arg/p2, normalize, transpose, PV
        # Emitting stage2(h-1) before stage1(h) keeps the ACT queue mono-
        # function per phase (all sqrt deps are ready a full head early), so
        # the activation table switches only ~2x per head.
        with tc.tile_pool(name="pe_", bufs=2) as pe_, \
             tc.tile_pool(name="pC", bufs=2) as pC, \
             tc.tile_pool(name="pdg", bufs=1) as pdg, \
             tc.tile_pool(name="p16s", bufs=1) as p16s, \
             tc.tile_pool(name="pt1", bufs=2) as pt1, \
             tc.tile_pool(name="parg", bufs=2) as parg, \
             tc.tile_pool(name="p16", bufs=2) as p16, \
             tc.tile_pool(name="p16n", bufs=2) as p16n, \
             tc.tile_pool(name="p16q", bufs=2) as p16q, \
             tc.tile_pool(name="pt", bufs=1) as pt, \
             tc.tile_pool(name="prg", bufs=2) as prg, \
             tc.tile_pool(name="tiny", bufs=24) as ptiny, \
             tc.tile_pool(name="ps_s", bufs=2, space="PSUM") as pps_s, \
             tc.tile_pool(name="ps_o", bufs=1, space="PSUM") as pps_o:

            cmt_m = pc.tile([128, 4, T], dt.float16)   # attention out, P4 layout
            cmt_c = pc.tile([128, 4, T], dt.float16)
            wo16 = pc.tile([128, 2, 4, D], dt.float16)
            nc.sync.dma_start(out=wo16[:, 0], in_=woT[0].rearrange("(k p) d -> p k d", p=128))
            nc.sync.dma_start(out=wo16[:, 1], in_=woT[1].rearrange("(k p) d -> p k d", p=128))

            om_m = om_c = None
            p2T = None
            sc16 = {0: {}, 1: {}}
            dg16 = {0: {}, 1: {}}
            rg8 = {}
            e_last = {}
            p2_last = {}
            q8gate = {}

            def stage1(h):
                g2 = float(gamma2[h])
                par = h % 2
                escale = 0.125
                rg8[par] = prg.tile([128, 8], dt.float32, tag="rg8", name="rg8")
                for tb in range(NT):
                    W = (tb + 1) * 128
                    d0 = tb * 128
                    ps = pps_s.tile([128, 1024], dt.float32, tag="ps_s")
                    nchunks = [(0, min(W, 512))] + ([(512, W)] if W > 512 else [])
                    for (s0, s1) in nchunks:
                        has_diag = s1 > d0
                        nc.tensor.matmul(ps[:, s0:s1], U2[:, h, ts(tb, 128)],
                                         W2[:, h, s0:s1], start=True, stop=False)
                        nc.tensor.matmul(ps[:, s0:s1], sel[:, h, :],
                                         sb8[:, s0:s1],
                                         start=False, stop=not has_diag)
                        if has_diag:
                            nc.tensor.matmul(ps[:, d0:W], eye_sb, btri_sb,
                                             start=False, stop=True)
                    e = pe_.tile([128, T], dt.float32, tag="e")
                    nc.scalar.activation(out=e[:, :W], in_=ps[:, :W], func=AF.Exp,
                                         scale=escale, bias=a1n8[:, tb, h:h + 1])
                    if tb == NT - 1:
                        e_last[h] = e
                    sc = p16s.tile([128, (tb + 1) * 128], dt.float16, tag=f"sc{par}{tb}", name="sc")
                    nc.vector.tensor_scalar(out=sc[:, :W], in0=ps[:, :W],
                                            scalar1=a1m8[:, tb, h:h + 1], scalar2=0.125,
                                            op0=OP.add, op1=OP.mult)
                    sc16[par][tb] = sc
                    C = pC.tile([128, T], dt.float32, tag="C")
                    nc.vector.tensor_tensor_scan(out=C[:, :W], data0=e[:, :W], data1=e[:, :W],
                                                 initial=0.0, op0=OP.add, op1=OP.bypass)
                    sm1 = C[:, W - 1:W]
                    rcp1 = ptiny.tile([128, 1], dt.float32, tag="rcp1")
                    nc.vector.reciprocal(out=rcp1, in_=sm1)
                    nc.vector.tensor_scalar(out=rg8[par][:, tb:tb + 1], in0=rcp1,
                                            scalar1=g2, scalar2=None, op0=OP.mult)
                    t1 = pt1.tile([128, T], dt.float32, tag="t1")
                    nc.vector.tensor_scalar(out=t1[:, :W], in0=C[:, :W], scalar1=sm1,
                                            scalar2=None, op0=OP.subtract)
                    dg = pdg.tile([128, (tb + 1) * 128], dt.float16, tag=f"dg{par}{tb}", name="dg")
                    nc.gpsimd.tensor_tensor(out=dg[:, :W], in0=t1[:, :W],
                                            in1=nneg_sb[:, NOFF[tb]:NOFF[tb] + W],
                                            op=OP.mult)
                    dg16[par][tb] = dg

            def stage2(h):
                nonlocal om_m, om_c, p2T
                par = h % 2
                # gate the sqrt batch on the full preceding Exp phase
                # (e(h) and p2(h-1)) so the ACT queue stays mono-function
                tok0 = ptiny.tile([128, 1], dt.float32, tag="tok0", name="tok0")
                if h >= 1:
                    nc.vector.scalar_tensor_tensor(out=tok0, in0=e_last[h][:, 0:1],
                                                   scalar=0.0, in1=p2_last[h - 1][:, 0:1],
                                                   op0=OP.mult, op1=OP.mult)
                else:
                    nc.vector.tensor_scalar(out=tok0, in0=e_last[h][:, 0:1],
                                            scalar1=0.0, scalar2=None, op0=OP.mult)
                rg8g = prg.tile([128, 8], dt.float32, tag="rg8g", name="rg8g")
                nc.vector.tensor_tensor(out=rg8g, in0=rg8[par],
                                        in1=bass.AP(tensor=tok0.tensor, offset=tok0.offset,
                                                    ap=[tok0.ap[0], [0, 8]]),
                                        op=OP.add)
                for tb in range(NT):
                    W = (tb + 1) * 128
                    nc.scalar.activation(out=dg16[par][tb][:, :W], in_=dg16[par][tb][:, :W],
                                         func=AF.Sqrt, scale=rg8g[:, tb:tb + 1])
                # -1 scale token: makes every te wait for the whole sqrt batch
                neg1 = ptiny.tile([128, 1], dt.float32, tag="neg1", name="neg1")
                nc.vector.tensor_scalar(out=neg1, in0=dg16[par][NT - 1][:, 0:1],
                                        scalar1=0.0, scalar2=-1.0, op0=OP.mult, op1=OP.add)
                # 0.125 scale token for the next head's e-batch (same gate)
                q8 = ptiny.tile([128, 1], dt.float32, tag="q8", name="q8")
                nc.vector.tensor_scalar(out=q8, in0=dg16[par][NT - 1][:, 0:1],
                                        scalar1=0.0, scalar2=0.125, op0=OP.mult, op1=OP.add)
                q8gate[h] = q8
                p2T = pt.tile([128, NT, T], dt.float16, tag="p2T")
                for tb in range(NT):
                    W = (tb + 1) * 128
                    nc.scalar.activation(out=dg16[par][tb][:, :W], in_=dg16[par][tb][:, :W],
                                         func=AF.Exp, scale=neg1)
                    arg = parg.tile([128, T], dt.float16, tag="arg")
                    nc.vector.tensor_tensor(out=arg[:, :W], in0=sc16[par][tb][:, :W],
                                            in1=dg16[par][tb][:, :W], op=OP.mult)
                    p2 = p16.tile([128, T], dt.float16, tag="p2")
                    sm2 = ptiny.tile([128, 1], dt.float32, tag="sm2")
                    nc.scalar.activation(out=p2[:, :W], in_=arg[:, :W], func=AF.Exp,
                                         accum_out=sm2)
                    if tb == NT - 1:
                        p2_last[h] = p2
                    rcp2 = ptiny.tile([128, 1], dt.float32, tag="rcp2")
                    nc.vector.reciprocal(out=rcp2, in_=sm2)
                    p2n = p16n.tile([128, T], dt.float16, tag="p2n")
                    nc.vector.tensor_scalar(out=p2n[:, :W], in0=p2[:, :W], scalar1=rcp2,
                                            scalar2=None, op0=OP.mult)
                    nc.sync.dma_start_transpose(out=p2T[:, 0:tb + 1, ts(tb, 128)],
                                                in_=p2n[:, :W])

                # ---- PV for head h (pair-shared psum) ----
                half_p = (h % 2) * 64
                if h % 2 == 0:
                    om_m = pps_o.tile([128, 1024], dt.float32, tag="om_m")
                    om_c = pps_o.tile([128, 1024], dt.float32, tag="om_c")
                hs = slice(h * DK, (h + 1) * DK)
                for j in range(NT):
                    w_j = (NT - j) * 128
                    tr = [(j * 128, 512), (512, 1024)] if j < 4 else [(j * 128, 1024)]
                    def _stop(t0, t1, j=j):
                        return (j == 3) if t1 <= 512 else (j == 7)
                    for (t0, t1) in tr:
                        nc.tensor.matmul(om_m[half_p:half_p + 64, t0:t1], vm16[:, j, hs],
                                         p2T[:, j, t0:t1],
                                         start=(j == 0), stop=_stop(t0, t1))
                    p2sq = p16q.tile([128, T], dt.float16, tag="p2sq")
                    nc.vector.tensor_tensor(out=p2sq[:, :w_j], in0=p2T[:, j, j * 128:1024],
                                            in1=p2T[:, j, j * 128:1024], op=OP.mult)
                    for (t0, t1) in tr:
                        nc.tensor.matmul(om_c[half_p:half_p + 64, t0:t1], vc16[:, j, hs],
                                         p2sq[:, t0 - j * 128: t1 - j * 128],
                                         start=(j == 0), stop=_stop(t0, t1))
                if h % 2 == 1:
                    pair = h // 2
                    nc.scalar.activation(out=cmt_m[:, pair, :], in_=om_m, func=AF.Identity,
                                         bias=bvp_sb[:, pair:pair + 1])
                    nc.vector.tensor_scalar(out=cmt_c[:, pair, :], in0=om_c,
                                            scalar1=bvp_sb[:, 4 + pair:5 + pair],
                                            scalar2=None, op0=OP.add)

            for h in range(H + 1):
                if h >= 1:
                    stage2(h - 1)
                if h < H:
                    stage1(h)

        # =================== P4: output projections ===================
        with tc.tile_pool(name="p4s", bufs=2) as p4s, \
             tc.tile_pool(name="p4r", bufs=1) as p4r, \
             tc.tile_pool(name="p4ps", bufs=2, space="PSUM") as p4ps:
            for i, (dst, cmt) in enumerate([(out_m, cmt_m), (out_c, cmt_c)]):
                borow = p4r.tile([1, D], dt.float16, tag="borow")
                nc.sync.dma_start(out=borow, in_=bvo16[i:i + 1, :])
                for m in range(NT):
                    ps = p4ps.tile([128, 512], dt.float32, tag="ps4")
                    for k in range(4):
                        nc.tensor.matmul(ps, cmt[:, k, ts(m, 128)], wo16[:, i, k, :],
                                         start=(k == 0), stop=False)
                    nc.tensor.matmul(ps, ones16[0:1, ts(m, 128)], borow,
                                     start=False, stop=True)
                    st = p4s.tile([128, 512], dt.float32, tag="st4")
                    nc.scalar.activation(out=st, in_=ps, func=AF.Copy)
                    nc.sync.dma_start(out=dst[ts(m, 128), :], in_=st)

    nc.finalize()
    return nc


def kernel(**inputs):
    f32 = lambda k: np.ascontiguousarray(np.asarray(inputs[k], np.float32))
    Wk, bk = f32('Wk_mean'), f32('bk_mean')
    Wkc, bkc = f32('Wk_cov'), f32('bk_cov')
    Wv, bv = f32('Wv_mean'), f32('bv_mean')
    Wvc, bvc = f32('Wv_cov'), f32('bv_cov')
    Wo, bo = f32('Wo_mean'), f32('bo_mean')
    Woc, boc = f32('Wo_cov'), f32('bo_cov')
    gammas = f32('gammas').reshape(H)
    zero_pad = int(np.asarray(inputs['zero_pad']))

    gamma = -np.log1p(np.exp(gammas))          # -softplus
    gamma2 = (gamma * gamma).astype(np.float64)

    # head-pair-swapped column permutation for the cov-side weights
    perm = np.arange(D).reshape(4, 2, DK)[:, ::-1, :].reshape(D)
    WkcT_sw = np.ascontiguousarray(Wkc.T[:, perm])
    bkc_sw = bkc[perm]

    wT = np.stack([np.ascontiguousarray(Wk.T), WkcT_sw,
                   np.ascontiguousarray(Wv.T), np.ascontiguousarray(Wvc.T)]).astype(F16)
    woT = np.stack([np.ascontiguousarray(Wo.T), np.ascontiguousarray(Woc.T)]).astype(F16)
    wc = np.ascontiguousarray(Wkc.T.reshape(D, H, DK).sum(-1)).astype(F16)  # [din, H]

    bqk = np.zeros((128, 12), np.float32)
    bqk[:, 0:4] = bk.reshape(4, 128).T
    bqk[:, 4:8] = bkc_sw.reshape(4, 128).T
    bqk[:, 8:12] = 2.0 * bk.reshape(4, 128).T
    bvo16 = np.stack([bo, boc]).astype(F16)
    bvp = np.concatenate([bv.reshape(4, 128).T, bvc.reshape(4, 128).T], axis=1).astype(np.float32)
    sb = bkc.reshape(H, DK).sum(-1)
    # a1 side carries both bias sums (q-side sb and k-side sb): -0.125 * 2sb
    cbr = np.ascontiguousarray((-0.25 * sb)[None, :]).astype(np.float32)  # [1, 8]

    btri = np.triu(np.full((128, 128), NEG16, np.float32), 1).astype(F16)
    eye = np.eye(128, dtype=F16)
    selp = np.zeros((8, 8, 128), F16)
    for _h in range(H):
        selp[_h, _h, :] = 1.0
    idx_t = np.arange(T)
    nneg = np.zeros((128, NN_COLS), np.float32)
    for tb in range(NT):
        tt = tb * 128 + np.arange(128)
        W = (tb + 1) * 128
        nneg[:, NOFF[tb]:NOFF[tb] + W] = -np.abs(tt[:, None] - idx_t[None, :W])
    nneg = nneg.astype(F16)

    xs = [f32('q_mean'), f32('q_cov'), f32('k_mean'), f32('k_cov'),
          f32('v_mean'), f32('v_cov')]

    nc = _build(gamma2)

    in_maps = []
    for b in range(B):
        xTb = np.stack([np.ascontiguousarray(x[b].T) for x in xs])
        in_maps.append(dict(xT=xTb, wT=wT, woT=woT, wc=wc, bqk=bqk, bvp=bvp,
                            bvo16=bvo16, cbr=cbr, btri=btri, eye=eye, selp=selp,
                            nneg=nneg))

    from concourse.bass_utils import run_bass_kernel_spmd
    trace = bool(int(os.environ.get("KERNEL_TRACE", "0")))
    kw = {}
    if os.environ.get("KERNEL_TMPDIR"):
        kw["tmpdir"] = os.environ["KERNEL_TMPDIR"]
    res = run_bass_kernel_spmd(nc, in_maps, list(range(B)), trace=trace, **kw)
    if trace and res.exec_time_ns is not None:
        print(f"HW exec time: {res.exec_time_ns} ns")
        if res.mean_exec_time_ns is not None:
            print(f"HW exec time mean: {res.mean_exec_time_ns:.0f} ns")

    out_mean = np.stack([res.results[b]["out_m"] for b in range(B)])
    out_cov = np.stack([res.results[b]["out_c"] for b in range(B)])
    if zero_pad:
        out_mean[:, 0, :] = bo[None, :]
        out_cov[:, 0, :] = boc[None, :]
    return out_mean, out_cov
